# revision 21
# baseline (speedup 1.0000x reference)
"""Multi-head causal attention (B=2, S=2048, E=1024, H=16) on 8 TRN2 cores.

Sharding: 2-way data parallel on batch x 4-way tensor parallel on heads.
Core c handles batch b = c//4 and heads [4g, 4g+4) where g = c%4.
Each core computes q/k/v projections for its 4 heads, causal attention,
and a partial output projection (row-parallel Wo slice); the host sums
the 4 partials per batch and adds bo.

v4 design:
- All matmul operands are bf16 (same 1 cycle/row PE rate as f32r, but
  half the DMA bytes / SBUF footprint). PSUM accumulation stays f32.
- The x stream is split across the sync and scalar DGE queues (one DMA
  per e-chunk); the weights move as three consolidated big-tile DMAs on
  the gpsimd (SWDGE) queue so their issue cost never delays x.
- Section-end softmax state is copied out of PSUM immediately (freeing
  the accumulation banks in ~2us) and the normalization multiplies are
  deferred into the filler stream, so neither the PSUM WAR hazard nor
  the broadcast-DMA latency ever stalls the PE or the DVE queue.
- Phase 1 projects six q/k tiles (both pairs of t=0, pair0 of t=1) and
  v chunks 0-3, with the six PSUM accumulators interleaved against the
  arriving x stream.
- The attention chunk loop is software-pipelined: QK^T + exp of chunk
  c+1 are emitted before P@V of chunk c, so the in-order PE never
  stalls on the Act-engine exp and holds its max p-state.
- Remaining projections and out-projection units are dripped into the
  PE stream at single-matmul granularity between attention chunks
  (deadline-scheduled, with a ready-delay for out-proj units and a
  small reserve kept for the tail).

Scores are computed transposed (k on partitions, q on free dim) so the
softmax denominator comes free as an extra ones-row in the P@V matmul,
and no P-tile transposes are needed anywhere.
"""

import sys

sys.path.insert(0, "/opt/trn_rl_repo")

from contextlib import ExitStack

import numpy as np

import concourse.bass as bass  # noqa: F401  (registers engines)
from concourse.ap import AP as _AP


def _free_bcast(src_ap, n):
    """View a [1, F] AP as [1, n, F] with a zero-stride middle dim (DMA replicate)."""
    return _AP(
        src_ap.tensor, src_ap.offset,
        [list(p) for p in src_ap.ap[:1]] + [[0, n]] + [list(p) for p in src_ap.ap[1:]],
    )

import concourse.tile as tile
from concourse import bacc, mybir
from concourse.bass_utils import run_bass_kernel_spmd

B, S, E, H = 2, 2048, 1024, 16
D = E // H            # 64
HPC = H // 4          # 4 heads per core
EC = HPC * D          # 256 = per-core head-dim width
NQT = S // 512        # 4 q-tiles of 512
NKC = S // 128        # 16 k-chunks of 128
NEC = E // 128        # 8 E-chunks of 128

F32 = mybir.dt.float32
BF16 = mybir.dt.bfloat16
EXP = mybir.ActivationFunctionType.Exp

# constants blob layout: [128, 961]
#   cols 0:896    staircase mask  M[kk, j] = 1.0 if j >= kk + 384 else 0
#   cols 896:898  ones, ones
#   cols 898:961  zeros
# (the all-ones regions of the staircase double as ones-vectors:
#  row 0 is ones on cols [384:896))
CST_W = 961

# v_sb per k-chunk: [128, 386]
#   h0: cols 0:64 v, 64 ones                 -> lhsT [0:65]   M=65  (sums row 64)
#   h1: col 65 ones, 66:129 zeros, 129:193 v -> lhsT [65:193] M=128 (sums row 0, data rows 64:128)
#   h2: cols 193:257 v, 257 ones             -> lhsT [193:258] M=65
#   h3: col 258 ones, 259:322 zeros, 322:386 v -> lhsT [258:386] M=128
V_W = 386
V_DATA = [0, 129, 193, 322]     # v data col start per local head
V_LHS = [(0, 65), (65, 193), (193, 258), (258, 386)]
V_STATIC = [64, 257]            # col starts of the [1,1,0*63] static blocks

# q/k tiles projected in phase 1 (in emission order); the rest are filler
PRE_QK = [(0, 0), (1, 0)]


def _build_nc():
    nc = bacc.Bacc("TRN2", target_bir_lowering=False, debug=False, num_devices=8)

    xT = nc.dram_tensor("xT", [E, S], BF16, kind="ExternalInput")
    wq = nc.dram_tensor("wq", [E, EC], BF16, kind="ExternalInput")
    wk = nc.dram_tensor("wk", [E, EC], BF16, kind="ExternalInput")
    wv = nc.dram_tensor("wv", [E, EC], BF16, kind="ExternalInput")
    wo = nc.dram_tensor("wo", [EC, E], BF16, kind="ExternalInput")
    bqd = nc.dram_tensor("bq", [EC], F32, kind="ExternalInput")
    bkd = nc.dram_tensor("bk", [EC], F32, kind="ExternalInput")
    bvd = nc.dram_tensor("bv", [EC], BF16, kind="ExternalInput")
    cst = nc.dram_tensor("cst", [128, CST_W], BF16, kind="ExternalInput")
    out = nc.dram_tensor("out", [S, E], BF16, kind="ExternalOutput")

    with tile.TileContext(nc) as tc:
        with ExitStack() as stack:
            cpool = stack.enter_context(tc.tile_pool(name="const", bufs=1))
            qkpool = stack.enter_context(tc.tile_pool(name="qkt", bufs=4))
            vpool = stack.enter_context(tc.tile_pool(name="vsb", bufs=NKC))
            wpool = stack.enter_context(tc.tile_pool(name="w", bufs=3))
            xpool = stack.enter_context(tc.tile_pool(name="xt", bufs=NEC))
            apool = stack.enter_context(tc.tile_pool(name="asb", bufs=2))
            ppool = stack.enter_context(tc.tile_pool(name="psb", bufs=3))
            rspool = stack.enter_context(tc.tile_pool(name="rs", bufs=2))
            bcpool = stack.enter_context(tc.tile_pool(name="bc", bufs=2))
            arpool = stack.enter_context(tc.tile_pool(name="ar", bufs=4))
            opool = stack.enter_context(tc.tile_pool(name="osb", bufs=4))

            # ---- constants + weights + input DMAs ----
            # sync queue: cst, x evens, biases; scalar queue: x odds;
            # gpsimd (SWDGE) queue: the consolidated weight DMAs.
            cst_sb = cpool.tile([128, CST_W], BF16, tag="cst")
            nc.gpsimd.dma_start(cst_sb[:], cst[:])
            static_blk = cst_sb[:, 896:961]        # [128,65] = [1,1,0*63]
            ones_row0 = cst_sb[0:1, 384:512]       # [1,128] ones at partition 0

            # per-queue DMA bandwidth is ~110 GB/s, so the ~6.5 MB of
            # input is balanced across all three DGE queues:
            #   sync:   wq, x0, x3, x6
            #   scalar: wk, x1, x4, x7
            #   gpsimd: cst, biases, wv, x2, x5, wo
            bq_sb = cpool.tile([128, 2], F32, tag="bq")
            nc.gpsimd.dma_start(bq_sb[:], bqd.ap().rearrange("(b p) -> p b", p=128))
            bk_sb = cpool.tile([128, 2], F32, tag="bk")
            nc.gpsimd.dma_start(bk_sb[:], bkd.ap().rearrange("(b p) -> p b", p=128))
            bv_sb = cpool.tile([1, EC], BF16, tag="bv")
            nc.gpsimd.dma_start(bv_sb[:], bvd.ap().rearrange("(o n) -> o n", o=1))

            w_sb = {}
            w_eng = {"q": nc.sync, "k": nc.scalar, "v": nc.gpsimd}
            for name, dram in (("q", wq), ("k", wk), ("v", wv)):
                t = wpool.tile([128, NEC * EC], BF16, tag=f"w{name}", name=f"w{name}")
                w_eng[name].dma_start(
                    t[:].rearrange("p (e n) -> p e n", e=NEC),
                    dram.ap().rearrange("(e p) n -> p e n", p=128),
                )
                w_sb[name] = t

            def w_lhs(name, e, pb):
                base = e * EC + pb * 128
                return w_sb[name][:, base:base + 128]

            x_eng = [nc.sync, nc.scalar, nc.gpsimd]
            xt_sb = []
            for e in range(NEC):
                t = xpool.tile([128, S], BF16, tag="xt", name=f"xt{e}")
                x_eng[e % 3].dma_start(t[:], xT[e * 128:(e + 1) * 128, :])
                xt_sb.append(t)
            wo_sb = []
            for j in range(2):
                t = cpool.tile([128, E], BF16, tag=f"wo{j}")
                nc.gpsimd.dma_start(t[:], wo[j * 128:(j + 1) * 128, :])
                wo_sb.append(t)

            # preload the exp table set (after the scalar-queue x issues)
            dummy = cpool.tile([1, 1], F32, tag="dummy")
            nc.scalar.activation(dummy[:], cst_sb[0:1, 0:1], EXP)

            # ---- persistent SBUF destinations ----
            # pair p rows: head 2p at partitions 0:64, head 2p+1 at 64:128
            qt_sb = [qkpool.tile([128, S], BF16, tag="qkt", name=f"qt{i}") for i in range(2)]
            kt_sb = [qkpool.tile([128, S], BF16, tag="qkt", name=f"kt{i}") for i in range(2)]
            v_sb = [vpool.tile([128, V_W], BF16, tag="vsb", name=f"v{m}") for m in range(NKC)]
            a_sb = [apool.tile([128, S], BF16, tag="asb", name=f"a{i}") for i in range(2)]
            bvb_sb = cpool.tile([128, EC], F32, tag="bvb")

            # ---- projection / out-projection generators ----
            # each yields after every matmul so the dripper can interleave at
            # single-matmul granularity
            def gen_qk_tile(name, dst, bias, pb, t, pool, tag):
                ps = pool.tile([128, 512], F32, tag=tag, name="qkps")
                for e in range(NEC):
                    nc.tensor.matmul(
                        ps[:],
                        w_lhs(name, e, pb),
                        xt_sb[e][:, t * 512:(t + 1) * 512],
                        start=(e == 0),
                        stop=(e == NEC - 1),
                    )
                    if e < NEC - 1:
                        yield
                nc.vector.tensor_scalar_add(
                    dst[pb][:, t * 512:(t + 1) * 512], ps[:], bias[:, pb:pb + 1]
                )
                yield

            def gen_v_chunk(m, pool, tag):
                vt = v_sb[m]
                for colstart in V_STATIC:
                    nc.gpsimd.tensor_copy(vt[:, colstart:colstart + 65], static_blk)
                vps = pool.tile([128, 512], F32, tag=tag, name="vps")
                for e in range(NEC):
                    nc.tensor.matmul(
                        vps[:, 0:EC],
                        xt_sb[e][:, m * 128:(m + 1) * 128],
                        w_sb["v"][:, e * EC:(e + 1) * EC],
                        start=(e == 0),
                        stop=(e == NEC - 1),
                    )
                    if e < NEC - 1:
                        yield
                for h in range(HPC):
                    d0 = V_DATA[h]
                    nc.vector.tensor_add(
                        vt[:, d0:d0 + 64],
                        vps[:, h * 64:(h + 1) * 64],
                        bvb_sb[:, h * 64:(h + 1) * 64],
                    )
                yield

            def gen_oproj(m, n, pool, tag):
                # out-proj unit: out[q,e] = sum_hd A[hd,q] Wo[hd,e]
                ops = pool.tile([128, 512], F32, tag=tag, name="ops")
                nc.tensor.matmul(
                    ops[:],
                    a_sb[0][:, m * 128:(m + 1) * 128],
                    wo_sb[0][:, n * 512:(n + 1) * 512],
                    start=True, stop=False,
                )
                yield
                nc.tensor.matmul(
                    ops[:],
                    a_sb[1][:, m * 128:(m + 1) * 128],
                    wo_sb[1][:, n * 512:(n + 1) * 512],
                    start=False, stop=True,
                )
                osb = opool.tile([128, 512], BF16, tag="osb", name="osb")
                (nc.vector.tensor_copy if (m + n) % 2 == 0 else nc.scalar.copy)(
                    osb[:], ops[:])
                (nc.sync if n == 0 else nc.gpsimd).dma_start(
                    out[m * 128:(m + 1) * 128, n * 512:(n + 1) * 512], osb[:]
                )
                yield

            # ---- phase 1: bvb + six q/k tiles + v chunks 0-3 ----
            pre_stack = ExitStack()
            pre_ps = pre_stack.enter_context(
                tc.tile_pool(name="pre_ps", bufs=7, space="PSUM"))

            # accumulators interleaved against the arriving x stream
            pre_gens = []
            for (pb, t) in PRE_QK:
                pre_gens.append(gen_qk_tile("q", qt_sb, bq_sb, pb, t, pre_ps, "pre"))
                pre_gens.append(gen_qk_tile("k", kt_sb, bk_sb, pb, t, pre_ps, "pre"))
            live = list(pre_gens)
            while live:
                live = [g for g in live if next(g, "done") != "done"]
            # bv broadcast [128, EC] = ones[1,128].T @ bv[1,EC] (emitted
            # after the q/k matmuls so a late bv DMA can't stall the PE)
            bvb_ps = pre_ps.tile([128, 512], F32, tag="pre", name="bvb_ps")
            nc.tensor.matmul(
                bvb_ps[:, 0:EC], ones_row0, bv_sb[:], start=True, stop=True
            )
            nc.vector.tensor_copy(bvb_sb[:], bvb_ps[:, 0:EC])
            for m in range(4):
                for _ in gen_v_chunk(m, pre_ps, "pre"):
                    pass
            pre_stack.close()

            # ---- phase 2: pipelined attention with dripped filler ----
            attn_stack = ExitStack()
            qk_ps = attn_stack.enter_context(
                tc.tile_pool(name="qk_ps", bufs=2, space="PSUM"))
            at_ps = attn_stack.enter_context(
                tc.tile_pool(name="at_ps", bufs=2, space="PSUM"))
            fill_ps = attn_stack.enter_context(
                tc.tile_pool(name="fill_ps", bufs=2, space="PSUM"))

            # flat job list: (pair, q-tile, chunk); chunk c covers k rows
            # [c*128, (c+1)*128) for q-tile t (q cols [t*512, (t+1)*512))
            jobs = []
            sec_start = {}
            for t in range(NQT):
                for p in range(2):
                    sec_start[(p, t)] = len(jobs)
                    for c in range(4 * (t + 1)):
                        jobs.append((p, t, c))
            njobs = len(jobs)

            # filler queue entries: [deadline, ready_after, generator]
            # deadline: all steps must be emitted before this job index runs;
            # ready_after: don't start stepping before this job index.
            filler = []
            for t in range(NQT):
                for p in range(2):
                    if (p, t) in PRE_QK:
                        continue
                    dl = sec_start[(p, t)]
                    filler.append([dl, 0, gen_qk_tile(
                        "q", qt_sb, bq_sb, p, t, fill_ps, "fill")])
                    filler.append([dl, 0, gen_qk_tile(
                        "k", kt_sb, bk_sb, p, t, fill_ps, "fill")])
                if t >= 1:
                    dl = sec_start[(0, t)]
                    for m in range(4 * t, 4 * (t + 1)):
                        filler.append([dl, 0, gen_v_chunk(m, fill_ps, "fill")])

            def drip(idx, steps):
                # deadline-forced items first (complete them fully) — the
                # correctness backstop for the in-order PE stream
                for ent in [f for f in filler if f[0] <= idx + 1]:
                    g = ent[2]
                    while next(g, "done") != "done":
                        pass
                    filler.remove(ent)
                # spread upcoming-deadline work over the preceding jobs so
                # it never lands as one burst that delays the next QK
                if any(f[0] <= idx + 4 for f in filler):
                    steps = max(steps, 5)
                # tail reserve: keep a few units back so the PE has work
                # while the final norm chain resolves
                if len(filler) <= 3 and idx < njobs - 12:
                    return
                while steps > 0 and filler:
                    ent = None
                    for cand in sorted(filler, key=lambda f: f[0]):
                        if cand[1] <= idx:
                            ent = cand
                            break
                    if ent is None:
                        return
                    if next(ent[2], "done") == "done":
                        filler.remove(ent)
                    steps -= 1

            # per-section live state
            sec_ps = {}    # (p, t) -> (ape, apo)
            sec_psb = {}   # job idx -> (psb, q0) for the pending PV

            def emit_qk_chunk(idx, p, t, c):
                d0 = c * 128 - t * 512
                # columns below d0 are fully masked: skip them on deep
                # diagonal chunks (d0>=256); shallow ones keep one wide exp
                q0 = d0 if d0 >= 256 else 0
                qsl = slice(t * 512 + q0, (t + 1) * 512)
                qkp = qk_ps.tile([128, 1024], F32, tag="qk", name="qkp")
                # scoresT [k-chunk, q-tile], both heads row-packed
                nc.tensor.matmul(
                    qkp[:, q0:512],
                    kt_sb[p][0:64, c * 128:(c + 1) * 128],
                    qt_sb[p][0:64, qsl],
                    start=True, stop=True,
                )
                nc.tensor.matmul(
                    qkp[:, 512 + q0:1024],
                    kt_sb[p][64:128, c * 128:(c + 1) * 128],
                    qt_sb[p][64:128, qsl],
                    start=True, stop=True,
                )
                psb = ppool.tile([128, 1024], BF16, tag="psb", name="psb")
                if q0 == 0:
                    nc.scalar.activation(psb[:], qkp[:], EXP)
                else:
                    nc.scalar.activation(psb[:, q0:512], qkp[:, q0:512], EXP)
                    nc.scalar.activation(
                        psb[:, 512 + q0:1024], qkp[:, 512 + q0:1024], EXP)
                if d0 >= 0:
                    off = 384 - d0
                    for hh in range(2):
                        nc.vector.tensor_mul(
                            psb[:, hh * 512 + q0:(hh + 1) * 512],
                            psb[:, hh * 512 + q0:(hh + 1) * 512],
                            cst_sb[:, off + q0:off + 512],
                        )
                sec_psb[idx] = (psb, q0)

            def emit_pv(idx, p, t, c):
                psb, q0 = sec_psb.pop(idx)
                ape, apo = sec_ps[(p, t)]
                lhs_e = V_LHS[2 * p]      # even head of the pair
                lhs_o = V_LHS[2 * p + 1]  # odd head
                nchunks = 4 * (t + 1)
                first, last = (c == 0), (c == nchunks - 1)
                nc.tensor.matmul(
                    ape[0:65, q0:512],
                    v_sb[c][:, lhs_e[0]:lhs_e[1]],
                    psb[:, q0:512],
                    start=first, stop=last,
                )
                nc.tensor.matmul(
                    apo[:, q0:512],
                    v_sb[c][:, lhs_o[0]:lhs_o[1]],
                    psb[:, 512 + q0:1024],
                    start=first, stop=last,
                )
                return last

            # [1,64] ones rows for the denominator broadcast matmuls; the
            # lhsT must share its base partition with the rhs row, so pull
            # ones from mask row 64 (ones at cols >= 448) and row 0
            ones64_p64 = cst_sb[64:65, 448:512]
            ones64_p0 = cst_sb[0:1, 384:448]

            def gen_finish_norm(p, t, are, aro, idx):
                # deferred normalization: broadcast the denominator rows
                # down 64 partitions with two tiny PE matmuls (no DMA on
                # the critical chain), then reciprocal + multiply on DVE
                bcp = fill_ps.tile([128, 512], F32, tag="fill", name="bcp")
                nc.tensor.matmul(
                    bcp[0:64, :], ones64_p64, are[64:65, :], start=True, stop=True)
                nc.tensor.matmul(
                    bcp[64:128, :], ones64_p0, aro[0:1, :], start=True, stop=True)
                yield
                rcp = bcpool.tile([128, 512], F32, tag="bc", name="rcp")
                nc.vector.reciprocal_approx_fast(out=rcp[:], in_=bcp[:])
                nc.vector.tensor_mul(
                    a_sb[p][0:64, t * 512:(t + 1) * 512],
                    are[0:64, :], rcp[0:64, :],
                )
                nc.vector.tensor_mul(
                    a_sb[p][64:128, t * 512:(t + 1) * 512],
                    aro[64:128, :], rcp[64:128, :],
                )
                if p == 1:
                    # both pairs of q-tile t normalized -> out-proj ready
                    for m in range(4 * t, 4 * (t + 1)):
                        for n in range(2):
                            filler.append(
                                [njobs, idx + 2, gen_oproj(m, n, fill_ps, "fill")])
                yield

            def emit_norm(idx, p, t):
                # copy the accumulators out of PSUM right away (frees the
                # banks for the next section); everything else is deferred
                ape, apo = sec_ps.pop((p, t))
                are = arpool.tile([128, 512], BF16, tag="ar", name="are")
                aro = arpool.tile([128, 512], BF16, tag="ar", name="aro")
                nc.vector.tensor_copy(are[0:65, :], ape[0:65, :])
                nc.vector.tensor_copy(aro[:], apo[:])
                filler.insert(0, [idx + 6, idx + 2,
                                  gen_finish_norm(p, t, are, aro, idx)])

            prev = None
            for idx, (p, t, c) in enumerate(jobs):
                if c == 0:
                    ape = at_ps.tile([128, 512], F32, tag="at", name="ape")
                    apo = at_ps.tile([128, 512], F32, tag="at", name="apo")
                    sec_ps[(p, t)] = (ape, apo)
                emit_qk_chunk(idx, p, t, c)
                boundary = False
                if prev is not None:
                    if emit_pv(*prev):
                        emit_norm(prev[0], prev[1], prev[2])
                        boundary = True
                drip(idx, 6 if boundary else 2)
                prev = (idx, p, t, c)
            emit_pv(*prev)
            emit_norm(prev[0], prev[1], prev[2])
            for _, _, g in filler:
                for _ in g:
                    pass
            filler.clear()
            attn_stack.close()

    nc.compile()
    return nc


_NC = None


def _get_nc():
    global _NC
    if _NC is None:
        _NC = _build_nc()
    return _NC


def _constants():
    from ml_dtypes import bfloat16
    kk = np.arange(128, dtype=np.int64)[:, None]
    jj = np.arange(896, dtype=np.int64)[None, :]
    cst = np.zeros((128, CST_W), dtype=np.float32)
    cst[:, 0:896] = (jj >= kk + 384).astype(np.float32)
    cst[:, 896] = 1.0
    cst[:, 897] = 1.0
    return cst.astype(bfloat16)


def _in_maps(inputs, Wq, bq, Wk, bk, Wv, bv, Wo, bo):
    from ml_dtypes import bfloat16
    cst = _constants()
    scale = np.float32(1.0 / np.sqrt(D))
    xT = [np.ascontiguousarray(inputs[b].T).astype(bfloat16) for b in range(B)]

    in_maps = []
    for c in range(8):
        b, g = divmod(c, 4)
        sl = slice(g * EC, (g + 1) * EC)
        in_maps.append({
            "xT": xT[b],
            "wq": (np.ascontiguousarray(Wq[:, sl]) * scale).astype(bfloat16),
            "bq": (bq[sl] * scale).astype(np.float32),
            "wk": np.ascontiguousarray(Wk[:, sl]).astype(bfloat16),
            "bk": bk[sl].astype(np.float32),
            "wv": np.ascontiguousarray(Wv[:, sl]).astype(bfloat16),
            "bv": bv[sl].astype(bfloat16),
            "wo": np.ascontiguousarray(Wo[sl, :]).astype(bfloat16),
            "cst": cst,
        })
    return in_maps


def kernel(inputs, Wq, bq, Wk, bk, Wv, bv, Wo, bo):
    inputs = np.asarray(inputs, dtype=np.float32)
    Wq = np.asarray(Wq, dtype=np.float32)
    Wk = np.asarray(Wk, dtype=np.float32)
    Wv = np.asarray(Wv, dtype=np.float32)
    Wo = np.asarray(Wo, dtype=np.float32)
    bq = np.asarray(bq, dtype=np.float32)
    bk = np.asarray(bk, dtype=np.float32)
    bv = np.asarray(bv, dtype=np.float32)
    bo = np.asarray(bo, dtype=np.float32)

    nc = _get_nc()
    in_maps = _in_maps(inputs, Wq, bq, Wk, bk, Wv, bv, Wo, bo)
    res = run_bass_kernel_spmd(nc, in_maps, list(range(8)))
    outs = [np.asarray(r["out"], dtype=np.float32) for r in res.results]
    full = np.empty((B, S, E), dtype=np.float32)
    for b in range(B):
        full[b] = outs[4 * b] + outs[4 * b + 1] + outs[4 * b + 2] + outs[4 * b + 3]
        full[b] += bo
    return full


# revision 22
# speedup vs baseline: 1.0180x; 1.0180x over previous
"""Multi-head causal attention (B=2, S=2048, E=1024, H=16) on 8 TRN2 cores.

Sharding: 2-way data parallel on batch x 4-way tensor parallel on heads.
Core c handles batch b = c//4 and heads [4g, 4g+4) where g = c%4.
Each core computes q/k/v projections for its 4 heads, causal attention,
and a partial output projection (row-parallel Wo slice); the host sums
the 4 partials per batch and adds bo.

v4 design:
- All matmul operands are bf16 (same 1 cycle/row PE rate as f32r, but
  half the DMA bytes / SBUF footprint). PSUM accumulation stays f32.
- The x stream is split across the sync and scalar DGE queues (one DMA
  per e-chunk); the weights move as three consolidated big-tile DMAs on
  the gpsimd (SWDGE) queue so their issue cost never delays x.
- Section-end softmax state is copied out of PSUM immediately (freeing
  the accumulation banks in ~2us) and the normalization multiplies are
  deferred into the filler stream, so neither the PSUM WAR hazard nor
  the broadcast-DMA latency ever stalls the PE or the DVE queue.
- Phase 1 projects six q/k tiles (both pairs of t=0, pair0 of t=1) and
  v chunks 0-3, with the six PSUM accumulators interleaved against the
  arriving x stream.
- The attention chunk loop is software-pipelined: QK^T + exp of chunk
  c+1 are emitted before P@V of chunk c, so the in-order PE never
  stalls on the Act-engine exp and holds its max p-state.
- Remaining projections and out-projection units are dripped into the
  PE stream at single-matmul granularity between attention chunks
  (deadline-scheduled, with a ready-delay for out-proj units and a
  small reserve kept for the tail).

Scores are computed transposed (k on partitions, q on free dim) so the
softmax denominator comes free as an extra ones-row in the P@V matmul,
and no P-tile transposes are needed anywhere.
"""

import sys

sys.path.insert(0, "/opt/trn_rl_repo")

from contextlib import ExitStack

import numpy as np

import concourse.bass as bass  # noqa: F401  (registers engines)
from concourse.ap import AP as _AP


def _free_bcast(src_ap, n):
    """View a [1, F] AP as [1, n, F] with a zero-stride middle dim (DMA replicate)."""
    return _AP(
        src_ap.tensor, src_ap.offset,
        [list(p) for p in src_ap.ap[:1]] + [[0, n]] + [list(p) for p in src_ap.ap[1:]],
    )

import concourse.tile as tile
from concourse import bacc, mybir
from concourse.bass_utils import run_bass_kernel_spmd

B, S, E, H = 2, 2048, 1024, 16
D = E // H            # 64
HPC = H // 4          # 4 heads per core
EC = HPC * D          # 256 = per-core head-dim width
NQT = S // 512        # 4 q-tiles of 512
NKC = S // 128        # 16 k-chunks of 128
NEC = E // 128        # 8 E-chunks of 128

F32 = mybir.dt.float32
BF16 = mybir.dt.bfloat16
EXP = mybir.ActivationFunctionType.Exp

# constants blob layout: [128, 961]
#   cols 0:896    staircase mask  M[kk, j] = 1.0 if j >= kk + 384 else 0
#   cols 896:898  ones, ones
#   cols 898:961  zeros
# (the all-ones regions of the staircase double as ones-vectors:
#  row 0 is ones on cols [384:896))
CST_W = 961

# v_sb per k-chunk: [128, 386]
#   h0: cols 0:64 v, 64 ones                 -> lhsT [0:65]   M=65  (sums row 64)
#   h1: col 65 ones, 66:129 zeros, 129:193 v -> lhsT [65:193] M=128 (sums row 0, data rows 64:128)
#   h2: cols 193:257 v, 257 ones             -> lhsT [193:258] M=65
#   h3: col 258 ones, 259:322 zeros, 322:386 v -> lhsT [258:386] M=128
V_W = 386
V_DATA = [0, 129, 193, 322]     # v data col start per local head
V_LHS = [(0, 65), (65, 193), (193, 258), (258, 386)]
V_STATIC = [64, 257]            # col starts of the [1,1,0*63] static blocks

# q/k tiles projected in phase 1 (in emission order); the rest are filler
PRE_QK = [(0, 0), (1, 0)]


def _build_nc():
    nc = bacc.Bacc("TRN2", target_bir_lowering=False, debug=False, num_devices=8)

    xT = nc.dram_tensor("xT", [E, S], BF16, kind="ExternalInput")
    wq = nc.dram_tensor("wq", [E, EC], BF16, kind="ExternalInput")
    wk = nc.dram_tensor("wk", [E, EC], BF16, kind="ExternalInput")
    wv = nc.dram_tensor("wv", [E, EC], BF16, kind="ExternalInput")
    wo = nc.dram_tensor("wo", [EC, E], BF16, kind="ExternalInput")
    bqd = nc.dram_tensor("bq", [EC], F32, kind="ExternalInput")
    bkd = nc.dram_tensor("bk", [EC], F32, kind="ExternalInput")
    bvd = nc.dram_tensor("bv", [EC], BF16, kind="ExternalInput")
    cst = nc.dram_tensor("cst", [128, CST_W], BF16, kind="ExternalInput")
    out = nc.dram_tensor("out", [S, E], BF16, kind="ExternalOutput")

    with tile.TileContext(nc) as tc:
        with ExitStack() as stack:
            cpool = stack.enter_context(tc.tile_pool(name="const", bufs=1))
            qkpool = stack.enter_context(tc.tile_pool(name="qkt", bufs=4))
            vpool = stack.enter_context(tc.tile_pool(name="vsb", bufs=NKC))
            wpool = stack.enter_context(tc.tile_pool(name="w", bufs=3))
            xpool = stack.enter_context(tc.tile_pool(name="xt", bufs=NEC))
            apool = stack.enter_context(tc.tile_pool(name="asb", bufs=2))
            ppool = stack.enter_context(tc.tile_pool(name="psb", bufs=3))
            rspool = stack.enter_context(tc.tile_pool(name="rs", bufs=2))
            bcpool = stack.enter_context(tc.tile_pool(name="bc", bufs=2))
            arpool = stack.enter_context(tc.tile_pool(name="ar", bufs=4))
            opool = stack.enter_context(tc.tile_pool(name="osb", bufs=4))

            # ---- constants + weights + input DMAs ----
            # sync queue: cst, x evens, biases; scalar queue: x odds;
            # gpsimd (SWDGE) queue: the consolidated weight DMAs.
            cst_sb = cpool.tile([128, CST_W], BF16, tag="cst")
            nc.sync.dma_start(cst_sb[:], cst[:])
            static_blk = cst_sb[:, 896:961]        # [128,65] = [1,1,0*63]
            ones_row0 = cst_sb[0:1, 384:512]       # [1,128] ones at partition 0

            # per-queue DMA bandwidth is ~110 GB/s, so the ~6.5 MB of
            # input is balanced across all three DGE queues:
            #   sync:   cst, wq, x0, x3, x6
            #   scalar: wk, x1, x4, x7
            #   gpsimd: wv, x2, x5, wo, biases
            bq_sb = cpool.tile([128, 2], F32, tag="bq")
            nc.gpsimd.dma_start(bq_sb[:], bqd.ap().rearrange("(b p) -> p b", p=128))
            bk_sb = cpool.tile([128, 2], F32, tag="bk")
            nc.gpsimd.dma_start(bk_sb[:], bkd.ap().rearrange("(b p) -> p b", p=128))
            bv_sb = cpool.tile([1, EC], BF16, tag="bv")
            nc.gpsimd.dma_start(bv_sb[:], bvd.ap().rearrange("(o n) -> o n", o=1))

            w_sb = {}
            w_eng = {"q": nc.sync, "k": nc.scalar, "v": nc.gpsimd}
            for name, dram in (("q", wq), ("k", wk), ("v", wv)):
                t = wpool.tile([128, NEC * EC], BF16, tag=f"w{name}", name=f"w{name}")
                w_eng[name].dma_start(
                    t[:].rearrange("p (e n) -> p e n", e=NEC),
                    dram.ap().rearrange("(e p) n -> p e n", p=128),
                )
                w_sb[name] = t

            def w_lhs(name, e, pb):
                base = e * EC + pb * 128
                return w_sb[name][:, base:base + 128]

            x_eng = [nc.sync, nc.scalar, nc.gpsimd]
            xt_sb = []
            for e in range(NEC):
                t = xpool.tile([128, S], BF16, tag="xt", name=f"xt{e}")
                x_eng[e % 3].dma_start(t[:], xT[e * 128:(e + 1) * 128, :])
                xt_sb.append(t)
            wo_sb = []
            for j in range(2):
                t = cpool.tile([128, E], BF16, tag=f"wo{j}")
                nc.gpsimd.dma_start(t[:], wo[j * 128:(j + 1) * 128, :])
                wo_sb.append(t)

            # preload the exp table set (after the scalar-queue x issues)
            dummy = cpool.tile([1, 1], F32, tag="dummy")
            nc.scalar.activation(dummy[:], cst_sb[0:1, 0:1], EXP)

            # ---- persistent SBUF destinations ----
            # pair p rows: head 2p at partitions 0:64, head 2p+1 at 64:128
            qt_sb = [qkpool.tile([128, S], BF16, tag="qkt", name=f"qt{i}") for i in range(2)]
            kt_sb = [qkpool.tile([128, S], BF16, tag="qkt", name=f"kt{i}") for i in range(2)]
            v_sb = [vpool.tile([128, V_W], BF16, tag="vsb", name=f"v{m}") for m in range(NKC)]
            a_sb = [apool.tile([128, S], BF16, tag="asb", name=f"a{i}") for i in range(2)]
            bvb_sb = cpool.tile([128, EC], F32, tag="bvb")

            # ---- projection / out-projection generators ----
            # each yields after every matmul so the dripper can interleave at
            # single-matmul granularity
            def gen_qk_tile(name, dst, bias, pb, t, pool, tag):
                ps = pool.tile([128, 512], F32, tag=tag, name="qkps")
                for e in range(NEC):
                    nc.tensor.matmul(
                        ps[:],
                        w_lhs(name, e, pb),
                        xt_sb[e][:, t * 512:(t + 1) * 512],
                        start=(e == 0),
                        stop=(e == NEC - 1),
                    )
                    if e < NEC - 1:
                        yield
                nc.vector.tensor_scalar_add(
                    dst[pb][:, t * 512:(t + 1) * 512], ps[:], bias[:, pb:pb + 1]
                )
                yield

            def gen_v_chunk(m, pool, tag):
                vt = v_sb[m]
                for colstart in V_STATIC:
                    nc.gpsimd.tensor_copy(vt[:, colstart:colstart + 65], static_blk)
                vps = pool.tile([128, 512], F32, tag=tag, name="vps")
                for e in range(NEC):
                    nc.tensor.matmul(
                        vps[:, 0:EC],
                        xt_sb[e][:, m * 128:(m + 1) * 128],
                        w_sb["v"][:, e * EC:(e + 1) * EC],
                        start=(e == 0),
                        stop=(e == NEC - 1),
                    )
                    if e < NEC - 1:
                        yield
                for h in range(HPC):
                    d0 = V_DATA[h]
                    nc.vector.tensor_add(
                        vt[:, d0:d0 + 64],
                        vps[:, h * 64:(h + 1) * 64],
                        bvb_sb[:, h * 64:(h + 1) * 64],
                    )
                yield

            def gen_oproj(m, n, pool, tag):
                # out-proj unit: out[q,e] = sum_hd A[hd,q] Wo[hd,e]
                ops = pool.tile([128, 512], F32, tag=tag, name="ops")
                nc.tensor.matmul(
                    ops[:],
                    a_sb[0][:, m * 128:(m + 1) * 128],
                    wo_sb[0][:, n * 512:(n + 1) * 512],
                    start=True, stop=False,
                )
                yield
                nc.tensor.matmul(
                    ops[:],
                    a_sb[1][:, m * 128:(m + 1) * 128],
                    wo_sb[1][:, n * 512:(n + 1) * 512],
                    start=False, stop=True,
                )
                osb = opool.tile([128, 512], BF16, tag="osb", name="osb")
                (nc.vector.tensor_copy if (m + n) % 2 == 0 else nc.scalar.copy)(
                    osb[:], ops[:])
                (nc.sync if n == 0 else nc.gpsimd).dma_start(
                    out[m * 128:(m + 1) * 128, n * 512:(n + 1) * 512], osb[:]
                )
                yield

            # ---- phase 1: bvb + six q/k tiles + v chunks 0-3 ----
            pre_stack = ExitStack()
            pre_ps = pre_stack.enter_context(
                tc.tile_pool(name="pre_ps", bufs=7, space="PSUM"))

            # accumulators interleaved against the arriving x stream
            pre_gens = []
            for (pb, t) in PRE_QK:
                pre_gens.append(gen_qk_tile("q", qt_sb, bq_sb, pb, t, pre_ps, "pre"))
                pre_gens.append(gen_qk_tile("k", kt_sb, bk_sb, pb, t, pre_ps, "pre"))
            live = list(pre_gens)
            while live:
                live = [g for g in live if next(g, "done") != "done"]
            # bv broadcast [128, EC] = ones[1,128].T @ bv[1,EC] (emitted
            # after the q/k matmuls so a late bv DMA can't stall the PE)
            bvb_ps = pre_ps.tile([128, 512], F32, tag="pre", name="bvb_ps")
            nc.tensor.matmul(
                bvb_ps[:, 0:EC], ones_row0, bv_sb[:], start=True, stop=True
            )
            nc.vector.tensor_copy(bvb_sb[:], bvb_ps[:, 0:EC])
            for m in range(4):
                for _ in gen_v_chunk(m, pre_ps, "pre"):
                    pass
            pre_stack.close()

            # ---- phase 2: pipelined attention with dripped filler ----
            attn_stack = ExitStack()
            qk_ps = attn_stack.enter_context(
                tc.tile_pool(name="qk_ps", bufs=2, space="PSUM"))
            at_ps = attn_stack.enter_context(
                tc.tile_pool(name="at_ps", bufs=2, space="PSUM"))
            fill_ps = attn_stack.enter_context(
                tc.tile_pool(name="fill_ps", bufs=2, space="PSUM"))

            # flat job list: (pair, q-tile, chunk); chunk c covers k rows
            # [c*128, (c+1)*128) for q-tile t (q cols [t*512, (t+1)*512))
            jobs = []
            sec_start = {}
            for t in range(NQT):
                for p in range(2):
                    sec_start[(p, t)] = len(jobs)
                    for c in range(4 * (t + 1)):
                        jobs.append((p, t, c))
            njobs = len(jobs)

            # filler queue entries: [deadline, ready_after, generator]
            # deadline: all steps must be emitted before this job index runs;
            # ready_after: don't start stepping before this job index.
            filler = []
            for t in range(NQT):
                for p in range(2):
                    if (p, t) in PRE_QK:
                        continue
                    dl = sec_start[(p, t)]
                    filler.append([dl, 0, gen_qk_tile(
                        "q", qt_sb, bq_sb, p, t, fill_ps, "fill")])
                    filler.append([dl, 0, gen_qk_tile(
                        "k", kt_sb, bk_sb, p, t, fill_ps, "fill")])
                if t >= 1:
                    dl = sec_start[(0, t)]
                    for m in range(4 * t, 4 * (t + 1)):
                        filler.append([dl, 0, gen_v_chunk(m, fill_ps, "fill")])

            def drip(idx, steps):
                # deadline-forced items first (complete them fully) — the
                # correctness backstop for the in-order PE stream
                for ent in [f for f in filler if f[0] <= idx + 1]:
                    g = ent[2]
                    while next(g, "done") != "done":
                        pass
                    filler.remove(ent)
                # spread upcoming-deadline work over the preceding jobs so
                # it never lands as one burst that delays the next QK
                if any(f[0] <= idx + 4 for f in filler):
                    steps = max(steps, 5)
                # tail reserve: keep a few units back so the PE has work
                # while the final norm chain resolves
                if len(filler) <= 3 and idx < njobs - 12:
                    return
                while steps > 0 and filler:
                    ent = None
                    for cand in sorted(filler, key=lambda f: f[0]):
                        if cand[1] <= idx:
                            ent = cand
                            break
                    if ent is None:
                        return
                    if next(ent[2], "done") == "done":
                        filler.remove(ent)
                    steps -= 1

            # per-section live state
            sec_ps = {}    # (p, t) -> (ape, apo)
            sec_psb = {}   # job idx -> (psb, q0) for the pending PV

            def emit_qk_chunk(idx, p, t, c):
                d0 = c * 128 - t * 512
                # columns below d0 are fully masked: skip them on deep
                # diagonal chunks (d0>=256); shallow ones keep one wide exp
                q0 = d0 if d0 >= 256 else 0
                qsl = slice(t * 512 + q0, (t + 1) * 512)
                qkp = qk_ps.tile([128, 1024], F32, tag="qk", name="qkp")
                # scoresT [k-chunk, q-tile], both heads row-packed
                nc.tensor.matmul(
                    qkp[:, q0:512],
                    kt_sb[p][0:64, c * 128:(c + 1) * 128],
                    qt_sb[p][0:64, qsl],
                    start=True, stop=True,
                )
                nc.tensor.matmul(
                    qkp[:, 512 + q0:1024],
                    kt_sb[p][64:128, c * 128:(c + 1) * 128],
                    qt_sb[p][64:128, qsl],
                    start=True, stop=True,
                )
                psb = ppool.tile([128, 1024], BF16, tag="psb", name="psb")
                if q0 == 0:
                    nc.scalar.activation(psb[:], qkp[:], EXP)
                else:
                    nc.scalar.activation(psb[:, q0:512], qkp[:, q0:512], EXP)
                    nc.scalar.activation(
                        psb[:, 512 + q0:1024], qkp[:, 512 + q0:1024], EXP)
                if d0 >= 0:
                    off = 384 - d0
                    for hh in range(2):
                        nc.vector.tensor_mul(
                            psb[:, hh * 512 + q0:(hh + 1) * 512],
                            psb[:, hh * 512 + q0:(hh + 1) * 512],
                            cst_sb[:, off + q0:off + 512],
                        )
                sec_psb[idx] = (psb, q0)

            def emit_pv(idx, p, t, c):
                psb, q0 = sec_psb.pop(idx)
                ape, apo = sec_ps[(p, t)]
                lhs_e = V_LHS[2 * p]      # even head of the pair
                lhs_o = V_LHS[2 * p + 1]  # odd head
                nchunks = 4 * (t + 1)
                first, last = (c == 0), (c == nchunks - 1)
                nc.tensor.matmul(
                    ape[0:65, q0:512],
                    v_sb[c][:, lhs_e[0]:lhs_e[1]],
                    psb[:, q0:512],
                    start=first, stop=last,
                )
                nc.tensor.matmul(
                    apo[:, q0:512],
                    v_sb[c][:, lhs_o[0]:lhs_o[1]],
                    psb[:, 512 + q0:1024],
                    start=first, stop=last,
                )
                return last

            # [1,64] ones rows for the denominator broadcast matmuls; the
            # lhsT must share its base partition with the rhs row, so pull
            # ones from mask row 64 (ones at cols >= 448) and row 0
            ones64_p64 = cst_sb[64:65, 448:512]
            ones64_p0 = cst_sb[0:1, 384:448]

            def gen_finish_norm(p, t, are, aro, idx):
                # deferred normalization: broadcast the denominator rows
                # down 64 partitions with two tiny PE matmuls (no DMA on
                # the critical chain), then reciprocal + multiply on DVE
                bcp = fill_ps.tile([128, 512], F32, tag="fill", name="bcp")
                nc.tensor.matmul(
                    bcp[0:64, :], ones64_p64, are[64:65, :], start=True, stop=True)
                nc.tensor.matmul(
                    bcp[64:128, :], ones64_p0, aro[0:1, :], start=True, stop=True)
                yield
                rcp = bcpool.tile([128, 512], F32, tag="bc", name="rcp")
                nc.vector.reciprocal_approx_fast(out=rcp[:], in_=bcp[:])
                nc.vector.tensor_mul(
                    a_sb[p][0:64, t * 512:(t + 1) * 512],
                    are[0:64, :], rcp[0:64, :],
                )
                nc.vector.tensor_mul(
                    a_sb[p][64:128, t * 512:(t + 1) * 512],
                    aro[64:128, :], rcp[64:128, :],
                )
                if p == 1:
                    # both pairs of q-tile t normalized -> out-proj ready
                    for m in range(4 * t, 4 * (t + 1)):
                        for n in range(2):
                            filler.append(
                                [njobs, idx + 2, gen_oproj(m, n, fill_ps, "fill")])
                yield

            def emit_norm(idx, p, t):
                # copy the accumulators out of PSUM right away (frees the
                # banks for the next section); everything else is deferred
                ape, apo = sec_ps.pop((p, t))
                are = arpool.tile([128, 512], BF16, tag="ar", name="are")
                aro = arpool.tile([128, 512], BF16, tag="ar", name="aro")
                nc.vector.tensor_copy(are[0:65, :], ape[0:65, :])
                nc.vector.tensor_copy(aro[:], apo[:])
                filler.insert(0, [idx + 6, idx + 2,
                                  gen_finish_norm(p, t, are, aro, idx)])

            prev = None
            for idx, (p, t, c) in enumerate(jobs):
                if c == 0:
                    ape = at_ps.tile([128, 512], F32, tag="at", name="ape")
                    apo = at_ps.tile([128, 512], F32, tag="at", name="apo")
                    sec_ps[(p, t)] = (ape, apo)
                emit_qk_chunk(idx, p, t, c)
                boundary = False
                if prev is not None:
                    if emit_pv(*prev):
                        emit_norm(prev[0], prev[1], prev[2])
                        boundary = True
                drip(idx, 6 if boundary else 2)
                prev = (idx, p, t, c)
            emit_pv(*prev)
            emit_norm(prev[0], prev[1], prev[2])
            for _, _, g in filler:
                for _ in g:
                    pass
            filler.clear()
            attn_stack.close()

    nc.compile()
    return nc


_NC = None


def _get_nc():
    global _NC
    if _NC is None:
        _NC = _build_nc()
    return _NC


def _constants():
    from ml_dtypes import bfloat16
    kk = np.arange(128, dtype=np.int64)[:, None]
    jj = np.arange(896, dtype=np.int64)[None, :]
    cst = np.zeros((128, CST_W), dtype=np.float32)
    cst[:, 0:896] = (jj >= kk + 384).astype(np.float32)
    cst[:, 896] = 1.0
    cst[:, 897] = 1.0
    return cst.astype(bfloat16)


def _in_maps(inputs, Wq, bq, Wk, bk, Wv, bv, Wo, bo):
    from ml_dtypes import bfloat16
    cst = _constants()
    scale = np.float32(1.0 / np.sqrt(D))
    xT = [np.ascontiguousarray(inputs[b].T).astype(bfloat16) for b in range(B)]

    in_maps = []
    for c in range(8):
        b, g = divmod(c, 4)
        sl = slice(g * EC, (g + 1) * EC)
        in_maps.append({
            "xT": xT[b],
            "wq": (np.ascontiguousarray(Wq[:, sl]) * scale).astype(bfloat16),
            "bq": (bq[sl] * scale).astype(np.float32),
            "wk": np.ascontiguousarray(Wk[:, sl]).astype(bfloat16),
            "bk": bk[sl].astype(np.float32),
            "wv": np.ascontiguousarray(Wv[:, sl]).astype(bfloat16),
            "bv": bv[sl].astype(bfloat16),
            "wo": np.ascontiguousarray(Wo[sl, :]).astype(bfloat16),
            "cst": cst,
        })
    return in_maps


def kernel(inputs, Wq, bq, Wk, bk, Wv, bv, Wo, bo):
    inputs = np.asarray(inputs, dtype=np.float32)
    Wq = np.asarray(Wq, dtype=np.float32)
    Wk = np.asarray(Wk, dtype=np.float32)
    Wv = np.asarray(Wv, dtype=np.float32)
    Wo = np.asarray(Wo, dtype=np.float32)
    bq = np.asarray(bq, dtype=np.float32)
    bk = np.asarray(bk, dtype=np.float32)
    bv = np.asarray(bv, dtype=np.float32)
    bo = np.asarray(bo, dtype=np.float32)

    nc = _get_nc()
    in_maps = _in_maps(inputs, Wq, bq, Wk, bk, Wv, bv, Wo, bo)
    res = run_bass_kernel_spmd(nc, in_maps, list(range(8)))
    outs = [np.asarray(r["out"], dtype=np.float32) for r in res.results]
    full = np.empty((B, S, E), dtype=np.float32)
    for b in range(B):
        full[b] = outs[4 * b] + outs[4 * b + 1] + outs[4 * b + 2] + outs[4 * b + 3]
        full[b] += bo
    return full
